# revision 22
# baseline (speedup 1.0000x reference)
"""CARAFE content-aware upsampling on 8 Trainium2 NeuronCores.

Strategy (data parallel, hint-compliant):
  8 cores = 4 batch images x 2 row-halves (32 low-res rows each, +2-row halo).
  Per core, fully fused pipeline in SBUF:
    A) y_down = conv1x1(x, w_down)+b_down        (PE, K=256 in 2 chunks)
    Z) zT = (w_out . x) transposed               (PE produces [col, ch] directly)
    B) enc = conv3x3(y_down, w_enc)              (PE, 9 shifted accum matmuls)
    C) mask = softmax over 25 taps (4 groups)    (PE transpose+group-sums via an
       augmented selector matmul, DVE reciprocal + normalize)
    D) out = sum_k zT[window] * mask  + b_out    (PE: per-row banded matmuls;
       banded mask matrix built by a DRAM-roundtrip diagonal scatter DMA)
  The final 1x1 conv (w_out) is folded BEFORE reassembly (z-trick): conv and
  reassembly commute since both are linear; this runs the big conv at low res
  and skips materializing the upsampled intermediate.

v3 scheduling notes (from NTFF traces):
  - all weights/biases packed host-side into 2 DRAM tensors -> 2 dma_starts
    (each dma_start costs ~700ns on its issuing engine queue)
  - PE order A, Z, B, C, D; Z's PSUM->SBUF copies split across
    vector/scalar/gpsimd while B runs on PE
  - mask scatter issues: vector (hh=0, same queue as normalize -> no sem
    wait) + gpsimd (hh=1); reloads batched 2-chunks on sync only; stage-D
    bias adds all on scalar so no queue blocks another's DMA chain
  - output staged bf16 (halves output DMA), converted to f32 host-side

Layouts:
  xs     [256, 36, 68]  zero-padded shard (rows h0-2..h1+2, cols -2..65)
  zT     [68, 36, 256]  col-on-partition transpose of z = w_out . x
  B_h    [68, 1280]     banded masks: B[w+j, w*20 + i*4 + p] = mask[h,w,i,j,p]
  out    [256, 64, 128] hi-res shard
"""

import sys
import functools
import numpy as np
from contextlib import ExitStack

for _p in ("/opt/trn_rl_repo",):
    if _p not in sys.path:
        sys.path.insert(0, _p)

import concourse.bass as bass
import concourse.bacc as bacc
import concourse.mybir as mybir
import concourse.tile as tile
from concourse.bass_utils import run_bass_kernel_spmd

NCORES = 8
FP = mybir.dt.float32
BF = mybir.dt.bfloat16
AF = mybir.ActivationFunctionType
ALU = mybir.AluOpType

# wpack column offsets
W_DT0, W_DT1, W_ET, W_OT0, W_OT1, W_SAUG = 0, 128, 256, 1156, 1412, 1668
W_COLS = 1772


def _ap(base, offset_delta, dims):
    return bass.AP(tensor=base.tensor, offset=base.offset + offset_delta, ap=dims)


@functools.lru_cache(maxsize=1)
def _build():
    nc = bacc.Bacc("TRN2", target_bir_lowering=False, debug=False, num_devices=NCORES)

    xs_d = nc.declare_dram_parameter("xs", [256, 36, 68], BF, isOutput=False)
    wpk_d = nc.declare_dram_parameter("wpk", [128, W_COLS], BF, isOutput=False)
    bpk_d = nc.declare_dram_parameter("bpk", [128, 6], FP, isOutput=False)
    out_d = nc.declare_dram_parameter("out", [256, 64, 128], BF, isOutput=True)

    with tile.TileContext(nc) as tc:
        with ExitStack() as ctx:
            const = ctx.enter_context(tc.tile_pool(name="const", bufs=1))
            big = ctx.enter_context(tc.tile_pool(name="big", bufs=1))
            opool = ctx.enter_context(tc.tile_pool(name="opool", bufs=3))
            dpool = ctx.enter_context(tc.tile_pool(name="dpool", bufs=1, space="DRAM"))

            # ---- loads: 4 dma_starts total on sync, weights first ----
            wpk = const.tile([128, W_COLS], BF)
            nc.sync.dma_start(out=wpk[:], in_=wpk_d[:])
            xa = big.tile([128, 36, 68], BF)
            xb = big.tile([128, 36, 68], BF)
            nc.sync.dma_start(out=xa[:], in_=xs_d[0:128])
            nc.sync.dma_start(out=xb[:], in_=xs_d[128:256])
            bpk = const.tile([128, 6], FP)
            nc.sync.dma_start(out=bpk[:], in_=bpk_d[:])

            wdt0 = wpk[:, W_DT0 : W_DT0 + 128]
            wdt1 = wpk[:, W_DT1 : W_DT1 + 128]
            wot0 = wpk[:, W_OT0 : W_OT0 + 256]
            wot1 = wpk[:, W_OT1 : W_OT1 + 256]
            saug = wpk[0:100, W_SAUG : W_SAUG + 104]
            bd = bpk[:, 0:1]
            be = bpk[0:100, 1:2]
            bo = bpk[:, 2:4]
            edge = bpk[:, 4:6]

            ydown = big.tile([128, 34, 66], BF)
            zt = big.tile([68, 36, 256], BF)
            expv = big.tile([100, 32, 64], BF)
            maskv = big.tile([128, 16, 100], BF)
            inv = big.tile([128, 16, 4], FP)

            # DRAM staging for the banded-mask scatter: one slot per h row so
            # scatters never wait on reloads (no slot reuse WAR chain).
            bstage_all = dpool.tile([32, 68, 1280], BF, name="bstage_all")
            zero_b = big.tile([68, 1280], BF)
            nc.vector.memset(zero_b[:], 0.0)
            # single dma zero-init: src repeats the zero tile 32x (stride-0)
            nc.gpsimd.dma_start(
                out=_ap(bstage_all[:], 0, [[1280, 68], [68 * 1280, 32], [1, 1280]]),
                in_=_ap(zero_b[:], 0, [[1280, 68], [0, 32], [1, 1280]]),
            )
            # all 32 banded-mask rows live in one SBUF tile; reloads fill
            # disjoint slices so the whole scatter pipeline runs ahead of PE.
            # partitions 0..35 hold band rows 0..35 (w-half 0 windows);
            # partitions 64..99 hold band rows 32..67 (w-half 1 windows) so
            # the two half-row matmuls land on disjoint PE row-groups.
            btX = big.tile([128, 32, 1280], BF)
            # zT columns 30..65 re-based at partition 64 (w-half 1 lhsT)
            zt2 = big.tile([128, 36, 256], BF)

            psC = ctx.enter_context(tc.tile_pool(name="psC", bufs=2, space="PSUM"))
            psum_ab = ExitStack()
            psAZ = psum_ab.enter_context(tc.tile_pool(name="psAZ", bufs=3, space="PSUM"))
            psB = psum_ab.enter_context(tc.tile_pool(name="psB", bufs=2, space="PSUM"))

            # ---- stage A: y_down [128ch, 34r, 66c] = w_down . x + b_down ----
            row_blocks = [(0, 6), (6, 12), (12, 18), (18, 24), (24, 30), (30, 34)]
            for bi, (r0, r1) in enumerate(row_blocks):
                nr = r1 - r0
                pa = psAZ.tile([128, 6, 66], FP, tag="AZ")
                nc.tensor.matmul(
                    pa[:, 0:nr, :], wdt0, xa[:, 1 + r0 : 1 + r1, 1:67],
                    start=True, stop=False,
                )
                nc.tensor.matmul(
                    pa[:, 0:nr, :], wdt1, xb[:, 1 + r0 : 1 + r1, 1:67],
                    start=False, stop=True,
                )
                if r0 == 0:
                    nc.vector.tensor_scalar(
                        ydown[:, 0:1, :], pa[:, 0:1, :], bd, edge[:, 0:1],
                        op0=ALU.add, op1=ALU.mult,
                    )
                    nc.scalar.add(ydown[:, 1:6, :], pa[:, 1:6, :], add=bd)
                elif r1 == 34:
                    nc.vector.tensor_scalar(
                        ydown[:, 33:34, :], pa[:, 3:4, :], bd, edge[:, 1:2],
                        op0=ALU.add, op1=ALU.mult,
                    )
                    nc.scalar.add(ydown[:, 30:33, :], pa[:, 0:3, :], add=bd)
                else:
                    if bi % 2 == 0:
                        nc.vector.tensor_scalar(
                            ydown[:, r0:r1, :], pa[:, 0:nr, :], bd, None,
                            op0=ALU.add,
                        )
                    else:
                        nc.scalar.add(ydown[:, r0:r1, :], pa[:, 0:nr, :], add=bd)
            # zero the w=-1 / w=64 columns (conv zero-padding semantics)
            nc.vector.memset(ydown[:, :, 0:1], 0.0)
            nc.vector.memset(ydown[:, :, 65:66], 0.0)

            # ---- stage Z: zT [68col, 36r, 256ch] = (w_out . x)^T ----
            # PSUM->SBUF copies split across vector/scalar so they drain
            # while stage B runs on the PE (gpsimd cannot read PSUM)
            zcopy_eng = [nc.vector, nc.scalar]
            for g in range(18):
                pz = psAZ.tile([68, 2, 256], FP, tag="AZ")
                for rr in range(2):
                    r = 2 * g + rr
                    nc.tensor.matmul(
                        pz[:, rr, :], xa[:, r, :], wot0, start=True, stop=False
                    )
                    nc.tensor.matmul(
                        pz[:, rr, :], xb[:, r, :], wot1, start=False, stop=True
                    )
                eng = zcopy_eng[g % 2]
                if eng is nc.scalar:
                    eng.copy(zt[:, 2 * g : 2 * g + 2, :], pz[:])
                else:
                    eng.tensor_copy(zt[:, 2 * g : 2 * g + 2, :], pz[:])

            # w-half-1 lhsT copy; gpsimd is idle until the first scatters
            nc.gpsimd.dma_start(out=zt2[64:100, :, :], in_=zt[32:68, :, :])

            # ---- stage B: enc -> exp(enc + b_enc) [100, 32, 64] ----
            for b4 in range(4):
                pb = psB.tile([100, 8, 64], FP, tag="B")
                k = 0
                for di in range(3):
                    for dj in range(3):
                        t = 3 * di + dj
                        nc.tensor.matmul(
                            pb[:],
                            wpk[:, W_ET + 100 * t : W_ET + 100 * (t + 1)],
                            ydown[:, di + 8 * b4 : di + 8 * b4 + 8, dj : dj + 64],
                            start=(k == 0), stop=(k == 8),
                        )
                        k += 1
                nc.scalar.activation(
                    expv[:, 8 * b4 : 8 * b4 + 8, :], pb[:], AF.Exp, bias=be
                )

            # ---- stage C: transpose + group sums + normalize -> maskv ----
            # scatter hh=0 on vector right after its own normalize (no sem
            # wait), hh=1 on gpsimd; reloads batched 2 chunks/4 rows on sync
            expf = expv[:].rearrange("p a b -> p (a b)")
            for kc in range(16):
                pc = psC.tile([128, 104], FP, tag="C")
                nc.tensor.matmul(
                    pc[:],
                    expf[:, 128 * kc : 128 * (kc + 1)],
                    saug,
                    start=True, stop=True,
                )
                nc.vector.reciprocal(inv[:, kc, :], pc[:, 100:104])
                inv_b = _ap(inv[:], kc * 4, [[64, 128], [0, 25], [1, 4]])
                nc.vector.tensor_tensor(
                    maskv[:, kc, :].rearrange("p (k q) -> p k q", q=4),
                    pc[:, 0:100].rearrange("p (k q) -> p k q", q=4),
                    inv_b,
                    op=ALU.mult,
                )
                for hh in range(2):
                    h = 2 * kc + hh
                    srcm = maskv[hh * 64 : hh * 64 + 64, kc, :]
                    dstm = _ap(
                        bstage_all[:], h * 68 * 1280,
                        [[1300, 64], [1280, 5], [1, 20]],
                    )
                    nc.gpsimd.dma_start(out=dstm, in_=srcm)
                if kc % 2 == 1:
                    k4 = kc // 2  # rows 4*k4 .. 4*k4+3
                    base = 4 * k4 * 68 * 1280
                    srcA = _ap(
                        bstage_all[:], base, [[1280, 36], [68 * 1280, 4], [1, 1280]]
                    )
                    srcB = _ap(
                        bstage_all[:],
                        base + 32 * 1280,
                        [[1280, 36], [68 * 1280, 4], [1, 1280]],
                    )
                    nc.sync.dma_start(
                        out=btX[0:36, 4 * k4 : 4 * k4 + 4, :], in_=srcA
                    )
                    nc.sync.dma_start(
                        out=btX[64:100, 4 * k4 : 4 * k4 + 4, :], in_=srcB
                    )

            # close A/Z/B psum pools to free banks for D
            psum_ab.close()
            psD = ctx.enter_context(tc.tile_pool(name="psD", bufs=3, space="PSUM"))

            # ---- stage D: banded reassembly + b_out ----
            # Each (h, ch-half) runs as 5 tap-passes x 2 concurrent half-row
            # matmuls on PE row-groups (0,0) and (64,0); the two w-halves
            # accumulate in separate PSUM banks of one 2-bank tile. All bias
            # adds on scalar; output DMAs batched 8 low-res rows on sync/scalar.
            obs = [None, None]
            for h in range(32):
                if h % 4 == 0:
                    obs[0] = opool.tile([128, 8, 64, 2], BF, tag="ob0", name="ob0")
                    obs[1] = opool.tile([128, 8, 64, 2], BF, tag="ob1", name="ob1")
                for half in range(2):
                    pd2 = psD.tile([128, 1024], FP, tag="D")
                    for i in range(5):
                        rhsA = _ap(
                            btX[:], h * 1280 + 4 * i, [[40960, 36], [20, 32], [1, 4]]
                        )
                        nc.tensor.matmul(
                            pd2[:, 0:128].rearrange("p (w q) -> p w q", q=4),
                            zt[0:36, h + i, 128 * half : 128 * half + 128],
                            rhsA,
                            start=(i == 0), stop=(i == 4),
                        )
                        rhsB = _ap(
                            btX[:],
                            64 * 40960 + h * 1280 + 32 * 20 + 4 * i,
                            [[40960, 36], [20, 32], [1, 4]],
                        )
                        nc.tensor.matmul(
                            pd2[:, 512:640].rearrange("p (w q) -> p w q", q=4),
                            zt2[64:100, h + i, 128 * half : 128 * half + 128],
                            rhsB,
                            start=(i == 0), stop=(i == 4),
                        )
                    ob = obs[half]
                    q = h % 4
                    for whalf in range(2):
                        pd_v = _ap(
                            pd2[:], 512 * whalf, [[1024, 128], [2, 2], [4, 32], [1, 2]]
                        )
                        dst = ob[:, 2 * q : 2 * q + 2, 32 * whalf : 32 * whalf + 32, :]
                        nc.scalar.add(dst, pd_v, add=bo[:, half : half + 1])
                if h % 4 == 3:
                    for half in range(2):
                        nc.sync.dma_start(
                            out=out_d[
                                128 * half : 128 * (half + 1),
                                2 * h - 6 : 2 * h + 2,
                                :,
                            ],
                            in_=obs[half][:].rearrange("p a w q -> p a (w q)"),
                        )

    nc.compile()
    return nc


def _host_prep(x, w_down, b_down, w_enc, b_enc, w_out, b_out):
    import ml_dtypes

    bft = ml_dtypes.bfloat16
    x = np.asarray(x, np.float32)
    xp = np.pad(x, [(0, 0), (0, 0), (2, 2), (2, 2)]).astype(bft)
    wdt = np.ascontiguousarray(np.asarray(w_down, np.float32)[:, :, 0, 0].T.astype(bft))
    wet = np.ascontiguousarray(
        np.asarray(w_enc, np.float32).transpose(1, 2, 3, 0).reshape(128, 9 * 100)
    ).astype(bft)
    wot = np.ascontiguousarray(np.asarray(w_out, np.float32)[:, :, 0, 0].T.astype(bft))
    # saug: permuted identity (e=(i5,j5,p4) -> e'=(j5,i5,p4)) + 4 group-sum cols
    saug = np.zeros((100, 104), bft)
    for i in range(5):
        for j in range(5):
            for p in range(4):
                saug[(i * 5 + j) * 4 + p, j * 20 + i * 4 + p] = 1.0
    for e in range(100):
        saug[e, 100 + e % 4] = 1.0
    wpk = np.zeros((128, W_COLS), bft)
    wpk[:, W_DT0 : W_DT0 + 128] = wdt[0:128]
    wpk[:, W_DT1 : W_DT1 + 128] = wdt[128:256]
    wpk[:, W_ET : W_ET + 900] = wet
    wpk[:, W_OT0 : W_OT0 + 256] = wot[0:128]
    wpk[:, W_OT1 : W_OT1 + 256] = wot[128:256]
    wpk[0:100, W_SAUG : W_SAUG + 104] = saug

    bd = np.asarray(b_down, np.float32).reshape(128)
    bev = np.asarray(b_enc, np.float32).reshape(100)
    bov = np.asarray(b_out, np.float32).reshape(256)
    in_maps = []
    for c in range(NCORES):
        n, hh = c // 2, c % 2
        xs = np.ascontiguousarray(xp[n, :, hh * 32 : hh * 32 + 36, :])
        bpk = np.zeros((128, 6), np.float32)
        bpk[:, 0] = bd
        bpk[0:100, 1] = bev
        bpk[:, 2] = bov[0:128]
        bpk[:, 3] = bov[128:256]
        bpk[:, 4] = 0.0 if hh == 0 else 1.0
        bpk[:, 5] = 0.0 if hh == 1 else 1.0
        in_maps.append(dict(xs=xs, wpk=wpk, bpk=bpk))
    return in_maps


last_exec_time_ns = None


def kernel(x, w_down, b_down, w_enc, b_enc, w_out, b_out):
    global last_exec_time_ns
    nc = _build()
    in_maps = _host_prep(x, w_down, b_down, w_enc, b_enc, w_out, b_out)
    res = run_bass_kernel_spmd(nc, in_maps, list(range(NCORES)))
    last_exec_time_ns = res.exec_time_ns
    out = np.empty((4, 256, 128, 128), np.float32)
    for c in range(NCORES):
        n, hh = c // 2, c % 2
        out[n, :, hh * 64 : (hh + 1) * 64, :] = np.asarray(
            res.results[c]["out"], dtype=np.float32
        )
    return out


# revision 24
# speedup vs baseline: 1.3758x; 1.3758x over previous
"""CARAFE content-aware upsampling on 8 Trainium2 NeuronCores.

Strategy (data parallel, hint-compliant):
  8 cores = 4 batch images x 2 row-halves (32 low-res rows each, +2-row halo).
  Per core, fully fused pipeline in SBUF:
    A) y_down = conv1x1(x, w_down)+b_down        (PE, K=256 in 2 chunks)
    Z) zT = (w_out . x) transposed               (PE produces [col, ch] directly)
    B) enc = conv3x3(y_down, w_enc)              (PE, 9 shifted accum matmuls)
    C) mask = softmax over 25 taps (4 groups)    (PE transpose+group-sums via an
       augmented selector matmul, DVE reciprocal + normalize)
    D) out = sum_k zT[window] * mask  + b_out    (PE: per-row banded matmuls;
       banded mask matrix built by a DRAM-roundtrip diagonal scatter DMA)
  The final 1x1 conv (w_out) is folded BEFORE reassembly (z-trick): conv and
  reassembly commute since both are linear; this runs the big conv at low res
  and skips materializing the upsampled intermediate.

v3 scheduling notes (from NTFF traces):
  - all weights/biases packed host-side into 2 DRAM tensors -> 2 dma_starts
    (each dma_start costs ~700ns on its issuing engine queue)
  - PE order A, Z, B, C, D; Z's PSUM->SBUF copies split across
    vector/scalar/gpsimd while B runs on PE
  - mask scatter issues: vector (hh=0, same queue as normalize -> no sem
    wait) + gpsimd (hh=1); reloads batched 2-chunks on sync only; stage-D
    bias adds all on scalar so no queue blocks another's DMA chain
  - output staged bf16 (halves output DMA), converted to f32 host-side

Layouts:
  xs     [256, 36, 68]  zero-padded shard (rows h0-2..h1+2, cols -2..65)
  zT     [68, 36, 256]  col-on-partition transpose of z = w_out . x
  B_h    [68, 1280]     banded masks: B[w+j, w*20 + i*4 + p] = mask[h,w,i,j,p]
  out    [256, 64, 128] hi-res shard
"""

import sys
import functools
import numpy as np
from contextlib import ExitStack

for _p in ("/opt/trn_rl_repo",):
    if _p not in sys.path:
        sys.path.insert(0, _p)

import concourse.bass as bass
import concourse.bacc as bacc
import concourse.mybir as mybir
import concourse.tile as tile
from concourse.bass_utils import run_bass_kernel_spmd

NCORES = 8
FP = mybir.dt.float32
BF = mybir.dt.bfloat16
AF = mybir.ActivationFunctionType
ALU = mybir.AluOpType

# wpack column offsets
W_DT0, W_DT1, W_ET, W_OT0, W_OT1, W_SAUG = 0, 128, 256, 1156, 1412, 1668
W_COLS = 1772


def _ap(base, offset_delta, dims):
    return bass.AP(tensor=base.tensor, offset=base.offset + offset_delta, ap=dims)


@functools.lru_cache(maxsize=1)
def _build():
    nc = bacc.Bacc("TRN2", target_bir_lowering=False, debug=False, num_devices=NCORES)

    xs_d = nc.declare_dram_parameter("xs", [256, 36, 68], BF, isOutput=False)
    wpk_d = nc.declare_dram_parameter("wpk", [128, W_COLS], BF, isOutput=False)
    bpk_d = nc.declare_dram_parameter("bpk", [128, 6], FP, isOutput=False)
    out_d = nc.declare_dram_parameter("out", [256, 64, 128], BF, isOutput=True)

    with tile.TileContext(nc) as tc:
        with ExitStack() as ctx:
            const = ctx.enter_context(tc.tile_pool(name="const", bufs=1))
            big = ctx.enter_context(tc.tile_pool(name="big", bufs=1))
            opool = ctx.enter_context(tc.tile_pool(name="opool", bufs=3))
            dpool = ctx.enter_context(tc.tile_pool(name="dpool", bufs=1, space="DRAM"))

            # ---- loads: 4 dma_starts total on sync, weights first ----
            wpk = const.tile([128, W_COLS], BF)
            nc.sync.dma_start(out=wpk[:], in_=wpk_d[:])
            xa = big.tile([128, 36, 68], BF)
            xb = big.tile([128, 36, 68], BF)
            nc.sync.dma_start(out=xa[:], in_=xs_d[0:128])
            nc.sync.dma_start(out=xb[:], in_=xs_d[128:256])
            bpk = const.tile([128, 6], FP)
            nc.sync.dma_start(out=bpk[:], in_=bpk_d[:])

            wdt0 = wpk[:, W_DT0 : W_DT0 + 128]
            wdt1 = wpk[:, W_DT1 : W_DT1 + 128]
            wot0 = wpk[:, W_OT0 : W_OT0 + 256]
            wot1 = wpk[:, W_OT1 : W_OT1 + 256]
            saug = wpk[0:100, W_SAUG : W_SAUG + 104]
            bd = bpk[:, 0:1]
            be = bpk[0:100, 1:2]
            bo = bpk[:, 2:4]
            edge = bpk[:, 4:6]

            ydown = big.tile([128, 34, 66], BF)
            zt = big.tile([68, 36, 256], BF)
            expv = big.tile([100, 32, 64], BF)
            maskv = big.tile([128, 16, 100], BF)
            inv = big.tile([128, 16, 4], FP)

            # DRAM staging for the banded-mask scatter: 16 rotating slots
            # (WAR slack of 8 chunks) zero-initialized by 8 spread dma_starts
            # (one giant init serializes on a single hw queue).
            bstage_all = dpool.tile([16, 68, 1280], BF, name="bstage_all")
            zero_b = big.tile([68, 1280], BF)
            nc.vector.memset(zero_b[:], 0.0)
            for s in range(8):
                nc.gpsimd.dma_start(
                    out=_ap(
                        bstage_all[:],
                        2 * s * 68 * 1280,
                        [[1280, 68], [68 * 1280, 2], [1, 1280]],
                    ),
                    in_=_ap(zero_b[:], 0, [[1280, 68], [0, 2], [1, 1280]]),
                )
            # all 32 banded-mask rows live in one SBUF tile; reloads fill
            # disjoint slices so the whole scatter pipeline runs ahead of PE.
            # partitions 0..35 hold band rows 0..35 (w-half 0 windows);
            # partitions 64..99 hold band rows 32..67 (w-half 1 windows) so
            # the two half-row matmuls land on disjoint PE row-groups.
            btX = big.tile([128, 32, 1280], BF)
            # zT columns 30..65 re-based at partition 64 (w-half 1 lhsT)
            zt2 = big.tile([128, 36, 256], BF)

            psC = ctx.enter_context(tc.tile_pool(name="psC", bufs=2, space="PSUM"))
            psum_ab = ExitStack()
            psAZ = psum_ab.enter_context(tc.tile_pool(name="psAZ", bufs=3, space="PSUM"))
            psB = psum_ab.enter_context(tc.tile_pool(name="psB", bufs=2, space="PSUM"))

            # ---- stage A: y_down [128ch, 34r, 66c] = w_down . x + b_down ----
            row_blocks = [(0, 6), (6, 12), (12, 18), (18, 24), (24, 30), (30, 34)]
            for bi, (r0, r1) in enumerate(row_blocks):
                nr = r1 - r0
                pa = psAZ.tile([128, 6, 66], FP, tag="AZ")
                nc.tensor.matmul(
                    pa[:, 0:nr, :], wdt0, xa[:, 1 + r0 : 1 + r1, 1:67],
                    start=True, stop=False,
                )
                nc.tensor.matmul(
                    pa[:, 0:nr, :], wdt1, xb[:, 1 + r0 : 1 + r1, 1:67],
                    start=False, stop=True,
                )
                if r0 == 0:
                    nc.vector.tensor_scalar(
                        ydown[:, 0:1, :], pa[:, 0:1, :], bd, edge[:, 0:1],
                        op0=ALU.add, op1=ALU.mult,
                    )
                    nc.scalar.add(ydown[:, 1:6, :], pa[:, 1:6, :], add=bd)
                elif r1 == 34:
                    nc.vector.tensor_scalar(
                        ydown[:, 33:34, :], pa[:, 3:4, :], bd, edge[:, 1:2],
                        op0=ALU.add, op1=ALU.mult,
                    )
                    nc.scalar.add(ydown[:, 30:33, :], pa[:, 0:3, :], add=bd)
                else:
                    if bi % 2 == 0:
                        nc.vector.tensor_scalar(
                            ydown[:, r0:r1, :], pa[:, 0:nr, :], bd, None,
                            op0=ALU.add,
                        )
                    else:
                        nc.scalar.add(ydown[:, r0:r1, :], pa[:, 0:nr, :], add=bd)
            # zero the w=-1 / w=64 columns (conv zero-padding semantics)
            nc.vector.memset(ydown[:, :, 0:1], 0.0)
            nc.vector.memset(ydown[:, :, 65:66], 0.0)

            # ---- stage Z: zT [68col, 36r, 256ch] = (w_out . x)^T ----
            # PSUM->SBUF copies split across vector/scalar so they drain
            # while stage B runs on the PE (gpsimd cannot read PSUM)
            zcopy_eng = [nc.vector, nc.scalar]
            for g in range(18):
                pz = psAZ.tile([68, 2, 256], FP, tag="AZ")
                for rr in range(2):
                    r = 2 * g + rr
                    nc.tensor.matmul(
                        pz[:, rr, :], xa[:, r, :], wot0, start=True, stop=False
                    )
                    nc.tensor.matmul(
                        pz[:, rr, :], xb[:, r, :], wot1, start=False, stop=True
                    )
                eng = zcopy_eng[g % 2]
                if eng is nc.scalar:
                    eng.copy(zt[:, 2 * g : 2 * g + 2, :], pz[:])
                else:
                    eng.tensor_copy(zt[:, 2 * g : 2 * g + 2, :], pz[:])

            # w-half-1 lhsT copy; gpsimd is idle until the first scatters
            nc.gpsimd.dma_start(out=zt2[64:100, :, :], in_=zt[32:68, :, :])

            # ---- stage B: enc -> exp(enc + b_enc) [100, 32, 64] ----
            for b4 in range(4):
                pb = psB.tile([100, 8, 64], FP, tag="B")
                k = 0
                for di in range(3):
                    for dj in range(3):
                        t = 3 * di + dj
                        nc.tensor.matmul(
                            pb[:],
                            wpk[:, W_ET + 100 * t : W_ET + 100 * (t + 1)],
                            ydown[:, di + 8 * b4 : di + 8 * b4 + 8, dj : dj + 64],
                            start=(k == 0), stop=(k == 8),
                        )
                        k += 1
                nc.scalar.activation(
                    expv[:, 8 * b4 : 8 * b4 + 8, :], pb[:], AF.Exp, bias=be
                )

            # ---- stage C: transpose + group sums + normalize -> maskv ----
            # scatter hh=0 on vector right after its own normalize (no sem
            # wait), hh=1 on gpsimd; reloads batched 2 chunks/4 rows on sync
            expf = expv[:].rearrange("p a b -> p (a b)")
            for kc in range(16):
                pc = psC.tile([128, 104], FP, tag="C")
                nc.tensor.matmul(
                    pc[:],
                    expf[:, 128 * kc : 128 * (kc + 1)],
                    saug,
                    start=True, stop=True,
                )
                nc.vector.reciprocal(inv[:, kc, :], pc[:, 100:104])
                inv_b = _ap(inv[:], kc * 4, [[64, 128], [0, 25], [1, 4]])
                nc.vector.tensor_tensor(
                    maskv[:, kc, :].rearrange("p (k q) -> p k q", q=4),
                    pc[:, 0:100].rearrange("p (k q) -> p k q", q=4),
                    inv_b,
                    op=ALU.mult,
                )
                for hh in range(2):
                    h = 2 * kc + hh
                    srcm = maskv[hh * 64 : hh * 64 + 64, kc, :]
                    dstm = _ap(
                        bstage_all[:], (h % 16) * 68 * 1280,
                        [[1300, 64], [1280, 5], [1, 20]],
                    )
                    seng = nc.gpsimd if hh == 0 else nc.scalar
                    seng.dma_start(out=dstm, in_=srcm)
                if kc % 2 == 1:
                    k4 = kc // 2  # rows 4*k4 .. 4*k4+3, 16 slots never wrap
                    base = ((4 * k4) % 16) * 68 * 1280
                    srcA = _ap(
                        bstage_all[:], base, [[1280, 36], [68 * 1280, 4], [1, 1280]]
                    )
                    srcB = _ap(
                        bstage_all[:],
                        base + 32 * 1280,
                        [[1280, 36], [68 * 1280, 4], [1, 1280]],
                    )
                    nc.sync.dma_start(
                        out=btX[0:36, 4 * k4 : 4 * k4 + 4, :], in_=srcA
                    )
                    nc.sync.dma_start(
                        out=btX[64:100, 4 * k4 : 4 * k4 + 4, :], in_=srcB
                    )

            # close A/Z/B psum pools to free banks for D
            psum_ab.close()
            psD = ctx.enter_context(tc.tile_pool(name="psD", bufs=3, space="PSUM"))

            # ---- stage D: banded reassembly + b_out ----
            # Each (h, ch-half) runs as 5 tap-passes x 2 concurrent half-row
            # matmuls on PE row-groups (0,0) and (64,0); the two w-halves
            # accumulate in separate PSUM banks of one 2-bank tile. All bias
            # adds on scalar; output DMAs batched 8 low-res rows on sync/scalar.
            obs = [None, None]
            for h in range(32):
                if h % 4 == 0:
                    obs[0] = opool.tile([128, 8, 64, 2], BF, tag="ob0", name="ob0")
                    obs[1] = opool.tile([128, 8, 64, 2], BF, tag="ob1", name="ob1")
                for half in range(2):
                    pd2 = psD.tile([128, 1024], FP, tag="D")
                    for i in range(5):
                        rhsA = _ap(
                            btX[:], h * 1280 + 4 * i, [[40960, 36], [20, 32], [1, 4]]
                        )
                        nc.tensor.matmul(
                            pd2[:, 0:128].rearrange("p (w q) -> p w q", q=4),
                            zt[0:36, h + i, 128 * half : 128 * half + 128],
                            rhsA,
                            start=(i == 0), stop=(i == 4),
                        )
                        rhsB = _ap(
                            btX[:],
                            64 * 40960 + h * 1280 + 32 * 20 + 4 * i,
                            [[40960, 36], [20, 32], [1, 4]],
                        )
                        nc.tensor.matmul(
                            pd2[:, 512:640].rearrange("p (w q) -> p w q", q=4),
                            zt2[64:100, h + i, 128 * half : 128 * half + 128],
                            rhsB,
                            start=(i == 0), stop=(i == 4),
                        )
                    ob = obs[half]
                    q = h % 4
                    for whalf in range(2):
                        pd_v = _ap(
                            pd2[:], 512 * whalf, [[1024, 128], [2, 2], [4, 32], [1, 2]]
                        )
                        dst = ob[:, 2 * q : 2 * q + 2, 32 * whalf : 32 * whalf + 32, :]
                        nc.scalar.add(dst, pd_v, add=bo[:, half : half + 1])
                if h % 4 == 3:
                    for half in range(2):
                        nc.sync.dma_start(
                            out=out_d[
                                128 * half : 128 * (half + 1),
                                2 * h - 6 : 2 * h + 2,
                                :,
                            ],
                            in_=obs[half][:].rearrange("p a w q -> p a (w q)"),
                        )

    nc.compile()
    return nc


def _host_prep(x, w_down, b_down, w_enc, b_enc, w_out, b_out):
    import ml_dtypes

    bft = ml_dtypes.bfloat16
    x = np.asarray(x, np.float32)
    xp = np.pad(x, [(0, 0), (0, 0), (2, 2), (2, 2)]).astype(bft)
    wdt = np.ascontiguousarray(np.asarray(w_down, np.float32)[:, :, 0, 0].T.astype(bft))
    wet = np.ascontiguousarray(
        np.asarray(w_enc, np.float32).transpose(1, 2, 3, 0).reshape(128, 9 * 100)
    ).astype(bft)
    wot = np.ascontiguousarray(np.asarray(w_out, np.float32)[:, :, 0, 0].T.astype(bft))
    # saug: permuted identity (e=(i5,j5,p4) -> e'=(j5,i5,p4)) + 4 group-sum cols
    saug = np.zeros((100, 104), bft)
    for i in range(5):
        for j in range(5):
            for p in range(4):
                saug[(i * 5 + j) * 4 + p, j * 20 + i * 4 + p] = 1.0
    for e in range(100):
        saug[e, 100 + e % 4] = 1.0
    wpk = np.zeros((128, W_COLS), bft)
    wpk[:, W_DT0 : W_DT0 + 128] = wdt[0:128]
    wpk[:, W_DT1 : W_DT1 + 128] = wdt[128:256]
    wpk[:, W_ET : W_ET + 900] = wet
    wpk[:, W_OT0 : W_OT0 + 256] = wot[0:128]
    wpk[:, W_OT1 : W_OT1 + 256] = wot[128:256]
    wpk[0:100, W_SAUG : W_SAUG + 104] = saug

    bd = np.asarray(b_down, np.float32).reshape(128)
    bev = np.asarray(b_enc, np.float32).reshape(100)
    bov = np.asarray(b_out, np.float32).reshape(256)
    in_maps = []
    for c in range(NCORES):
        n, hh = c // 2, c % 2
        xs = np.ascontiguousarray(xp[n, :, hh * 32 : hh * 32 + 36, :])
        bpk = np.zeros((128, 6), np.float32)
        bpk[:, 0] = bd
        bpk[0:100, 1] = bev
        bpk[:, 2] = bov[0:128]
        bpk[:, 3] = bov[128:256]
        bpk[:, 4] = 0.0 if hh == 0 else 1.0
        bpk[:, 5] = 0.0 if hh == 1 else 1.0
        in_maps.append(dict(xs=xs, wpk=wpk, bpk=bpk))
    return in_maps


last_exec_time_ns = None


def kernel(x, w_down, b_down, w_enc, b_enc, w_out, b_out):
    global last_exec_time_ns
    nc = _build()
    in_maps = _host_prep(x, w_down, b_down, w_enc, b_enc, w_out, b_out)
    res = run_bass_kernel_spmd(nc, in_maps, list(range(NCORES)))
    last_exec_time_ns = res.exec_time_ns
    out = np.empty((4, 256, 128, 128), np.float32)
    for c in range(NCORES):
        n, hh = c // 2, c % 2
        out[n, :, hh * 64 : (hh + 1) * 64, :] = np.asarray(
            res.results[c]["out"], dtype=np.float32
        )
    return out


# revision 29
# speedup vs baseline: 1.4215x; 1.0332x over previous
"""CARAFE content-aware upsampling on 8 Trainium2 NeuronCores.

Strategy (data parallel, hint-compliant):
  8 cores = 4 batch images x 2 row-halves (32 low-res rows each, +2-row halo).
  Per core, fully fused pipeline in SBUF:
    A) y_down = conv1x1(x, w_down)+b_down        (PE, K=256 in 2 chunks)
    Z) zT = (w_out . x) transposed               (PE produces [col, ch] directly)
    B) enc = conv3x3(y_down, w_enc)              (PE, 9 shifted accum matmuls)
    C) mask = softmax over 25 taps (4 groups)    (PE transpose+group-sums via an
       augmented selector matmul, DVE reciprocal + normalize)
    D) out = sum_k zT[window] * mask  + b_out    (PE: per-row banded matmuls;
       banded mask matrix built by a DRAM-roundtrip diagonal scatter DMA)
  The final 1x1 conv (w_out) is folded BEFORE reassembly (z-trick): conv and
  reassembly commute since both are linear; this runs the big conv at low res
  and skips materializing the upsampled intermediate.

v3 scheduling notes (from NTFF traces):
  - all weights/biases packed host-side into 2 DRAM tensors -> 2 dma_starts
    (each dma_start costs ~700ns on its issuing engine queue)
  - PE order A, Z, B, C, D; Z's PSUM->SBUF copies split across
    vector/scalar/gpsimd while B runs on PE
  - mask scatter issues: vector (hh=0, same queue as normalize -> no sem
    wait) + gpsimd (hh=1); reloads batched 2-chunks on sync only; stage-D
    bias adds all on scalar so no queue blocks another's DMA chain
  - output staged bf16 (halves output DMA), converted to f32 host-side

Layouts:
  xs     [256, 36, 68]  zero-padded shard (rows h0-2..h1+2, cols -2..65)
  zT     [68, 36, 256]  col-on-partition transpose of z = w_out . x
  B_h    [68, 1280]     banded masks: B[w+j, w*20 + i*4 + p] = mask[h,w,i,j,p]
  out    [256, 64, 128] hi-res shard
"""

import sys
import functools
import numpy as np
from contextlib import ExitStack

for _p in ("/opt/trn_rl_repo",):
    if _p not in sys.path:
        sys.path.insert(0, _p)

import concourse.bass as bass
import concourse.bacc as bacc
import concourse.mybir as mybir
import concourse.tile as tile
from concourse.bass_utils import run_bass_kernel_spmd

NCORES = 8
FP = mybir.dt.float32
BF = mybir.dt.bfloat16
AF = mybir.ActivationFunctionType
ALU = mybir.AluOpType

# wpack column offsets
W_DT0, W_DT1, W_ET, W_OT0, W_OT1, W_SAUG = 0, 128, 256, 1156, 1412, 1668
W_COLS = 1772


def _ap(base, offset_delta, dims):
    return bass.AP(tensor=base.tensor, offset=base.offset + offset_delta, ap=dims)


@functools.lru_cache(maxsize=1)
def _build():
    nc = bacc.Bacc("TRN2", target_bir_lowering=False, debug=False, num_devices=NCORES)

    xs_d = nc.declare_dram_parameter("xs", [256, 36, 68], BF, isOutput=False)
    wpk_d = nc.declare_dram_parameter("wpk", [128, W_COLS], BF, isOutput=False)
    bpk_d = nc.declare_dram_parameter("bpk", [128, 6], FP, isOutput=False)
    out_d = nc.declare_dram_parameter("out", [256, 64, 128], BF, isOutput=True)

    with tile.TileContext(nc) as tc:
        with ExitStack() as ctx:
            const = ctx.enter_context(tc.tile_pool(name="const", bufs=1))
            big = ctx.enter_context(tc.tile_pool(name="big", bufs=1))
            opool = ctx.enter_context(tc.tile_pool(name="opool", bufs=3))
            dpool = ctx.enter_context(tc.tile_pool(name="dpool", bufs=1, space="DRAM"))

            # ---- loads on sync: w_down first, inputs split for fast arrival ----
            wpk = const.tile([128, W_COLS], BF)
            nc.sync.dma_start(out=wpk[:, 0:256], in_=wpk_d[:, 0:256])
            xa = big.tile([128, 36, 68], BF)
            xb = big.tile([128, 36, 68], BF)
            nc.sync.dma_start(out=xa[:, 0:18, :], in_=xs_d[0:128, 0:18, :])
            nc.sync.dma_start(out=xa[:, 18:36, :], in_=xs_d[0:128, 18:36, :])
            nc.sync.dma_start(out=xb[:, 0:18, :], in_=xs_d[128:256, 0:18, :])
            nc.sync.dma_start(out=xb[:, 18:36, :], in_=xs_d[128:256, 18:36, :])
            bpk = const.tile([128, 6], FP)
            nc.sync.dma_start(out=bpk[:], in_=bpk_d[:])
            nc.sync.dma_start(out=wpk[:, 256:W_COLS], in_=wpk_d[:, 256:W_COLS])

            wdt0 = wpk[:, W_DT0 : W_DT0 + 128]
            wdt1 = wpk[:, W_DT1 : W_DT1 + 128]
            wot0 = wpk[:, W_OT0 : W_OT0 + 256]
            wot1 = wpk[:, W_OT1 : W_OT1 + 256]
            saug = wpk[0:100, W_SAUG : W_SAUG + 104]
            bd = bpk[:, 0:1]
            be = bpk[0:100, 1:2]
            bo = bpk[:, 2:4]
            edge = bpk[:, 4:6]

            ydown = big.tile([128, 34, 66], BF)
            zt = big.tile([68, 36, 256], BF)
            expv = big.tile([100, 32, 64], BF)
            maskv = big.tile([128, 16, 100], BF)
            inv = big.tile([128, 16, 4], FP)

            # DRAM staging for the banded-mask scatter: 16 rotating slots
            # (WAR slack of 8 chunks) zero-initialized by 8 spread dma_starts
            # (one giant init serializes on a single hw queue).
            bstage_all = dpool.tile([16, 68, 1280], BF, name="bstage_all")
            zero_b = big.tile([68, 1280], BF)
            nc.vector.memset(zero_b[:], 0.0)
            # zero only the regions reloads read: rows 0:36 x cols 0:640
            # (w-half 0) and rows 32:68 x cols 640:1280 (w-half 1); all
            # scatter writes land inside this union.
            for s in range(8):
                nc.gpsimd.dma_start(
                    out=_ap(
                        bstage_all[:],
                        2 * s * 68 * 1280,
                        [[1280, 36], [68 * 1280, 2], [1, 640]],
                    ),
                    in_=_ap(zero_b[:], 0, [[1280, 36], [0, 2], [1, 640]]),
                )
                nc.gpsimd.dma_start(
                    out=_ap(
                        bstage_all[:],
                        2 * s * 68 * 1280 + 32 * 1280 + 640,
                        [[1280, 36], [68 * 1280, 2], [1, 640]],
                    ),
                    in_=_ap(zero_b[:], 0, [[1280, 36], [0, 2], [1, 640]]),
                )
            # all 32 banded-mask rows live in one SBUF tile; reloads fill
            # disjoint slices so the whole scatter pipeline runs ahead of PE.
            # partitions 0..35 hold band rows 0..35 (w-half 0 windows);
            # partitions 64..99 hold band rows 32..67 (w-half 1 windows) so
            # the two half-row matmuls land on disjoint PE row-groups.
            btX = big.tile([128, 32, 640], BF)
            # zT columns 30..65 re-based at partition 64 (w-half 1 lhsT)
            zt2 = big.tile([128, 36, 256], BF)

            psC = ctx.enter_context(tc.tile_pool(name="psC", bufs=2, space="PSUM"))
            psum_ab = ExitStack()
            psAZ = psum_ab.enter_context(tc.tile_pool(name="psAZ", bufs=3, space="PSUM"))
            psB = psum_ab.enter_context(tc.tile_pool(name="psB", bufs=2, space="PSUM"))

            # ---- stage A: y_down [128ch, 34r, 66c] = w_down . x + b_down ----
            row_blocks = [(0, 6), (6, 12), (12, 18), (18, 24), (24, 30), (30, 34)]
            for bi, (r0, r1) in enumerate(row_blocks):
                nr = r1 - r0
                pa = psAZ.tile([128, 6, 66], FP, tag="AZ")
                nc.tensor.matmul(
                    pa[:, 0:nr, :], wdt0, xa[:, 1 + r0 : 1 + r1, 1:67],
                    start=True, stop=False,
                )
                nc.tensor.matmul(
                    pa[:, 0:nr, :], wdt1, xb[:, 1 + r0 : 1 + r1, 1:67],
                    start=False, stop=True,
                )
                if r0 == 0:
                    nc.vector.tensor_scalar(
                        ydown[:, 0:1, :], pa[:, 0:1, :], bd, edge[:, 0:1],
                        op0=ALU.add, op1=ALU.mult,
                    )
                    nc.scalar.add(ydown[:, 1:6, :], pa[:, 1:6, :], add=bd)
                elif r1 == 34:
                    nc.vector.tensor_scalar(
                        ydown[:, 33:34, :], pa[:, 3:4, :], bd, edge[:, 1:2],
                        op0=ALU.add, op1=ALU.mult,
                    )
                    nc.scalar.add(ydown[:, 30:33, :], pa[:, 0:3, :], add=bd)
                else:
                    if bi % 2 == 0:
                        nc.vector.tensor_scalar(
                            ydown[:, r0:r1, :], pa[:, 0:nr, :], bd, None,
                            op0=ALU.add,
                        )
                    else:
                        nc.scalar.add(ydown[:, r0:r1, :], pa[:, 0:nr, :], add=bd)
            # zero the w=-1 / w=64 columns (conv zero-padding semantics)
            nc.vector.memset(ydown[:, :, 0:1], 0.0)
            nc.vector.memset(ydown[:, :, 65:66], 0.0)

            # ---- stage Z: zT [68col, 36r, 256ch] = (w_out . x)^T ----
            # PSUM->SBUF copies split across vector/scalar so they drain
            # while stage B runs on the PE (gpsimd cannot read PSUM)
            zcopy_eng = [nc.vector, nc.scalar]
            for g in range(18):
                pz = psAZ.tile([68, 2, 256], FP, tag="AZ")
                for rr in range(2):
                    r = 2 * g + rr
                    nc.tensor.matmul(
                        pz[:, rr, :], xa[:, r, :], wot0, start=True, stop=False
                    )
                    nc.tensor.matmul(
                        pz[:, rr, :], xb[:, r, :], wot1, start=False, stop=True
                    )
                eng = zcopy_eng[g % 2]
                if eng is nc.scalar:
                    eng.copy(zt[:, 2 * g : 2 * g + 2, :], pz[:])
                else:
                    eng.tensor_copy(zt[:, 2 * g : 2 * g + 2, :], pz[:])

            # w-half-1 lhsT copy; gpsimd is idle until the first scatters
            nc.gpsimd.dma_start(out=zt2[64:100, :, :], in_=zt[32:68, :, :])

            # ---- stage B: enc -> exp(enc + b_enc) [100, 32, 64] ----
            for b4 in range(4):
                pb = psB.tile([100, 8, 64], FP, tag="B")
                k = 0
                for di in range(3):
                    for dj in range(3):
                        t = 3 * di + dj
                        nc.tensor.matmul(
                            pb[:],
                            wpk[:, W_ET + 100 * t : W_ET + 100 * (t + 1)],
                            ydown[:, di + 8 * b4 : di + 8 * b4 + 8, dj : dj + 64],
                            start=(k == 0), stop=(k == 8),
                        )
                        k += 1
                nc.scalar.activation(
                    expv[:, 8 * b4 : 8 * b4 + 8, :], pb[:], AF.Exp, bias=be
                )

            # ---- stage C: transpose + group sums + normalize -> maskv ----
            # scatter hh=0 on vector right after its own normalize (no sem
            # wait), hh=1 on gpsimd; reloads batched 2 chunks/4 rows on sync
            expf = expv[:].rearrange("p a b -> p (a b)")
            for kc in range(16):
                pc = psC.tile([128, 104], FP, tag="C")
                nc.tensor.matmul(
                    pc[:],
                    expf[:, 128 * kc : 128 * (kc + 1)],
                    saug,
                    start=True, stop=True,
                )
                nc.vector.reciprocal(inv[:, kc, :], pc[:, 100:104])
                inv_b = _ap(inv[:], kc * 4, [[64, 128], [0, 25], [1, 4]])
                nc.vector.tensor_tensor(
                    maskv[:, kc, :].rearrange("p (k q) -> p k q", q=4),
                    pc[:, 0:100].rearrange("p (k q) -> p k q", q=4),
                    inv_b,
                    op=ALU.mult,
                )
                for hh in range(2):
                    h = 2 * kc + hh
                    srcm = maskv[hh * 64 : hh * 64 + 64, kc, :]
                    dstm = _ap(
                        bstage_all[:], (h % 16) * 68 * 1280,
                        [[1300, 64], [1280, 5], [1, 20]],
                    )
                    seng = nc.gpsimd if hh == 0 else nc.sync
                    seng.dma_start(out=dstm, in_=srcm)
                if kc % 2 == 1:
                    k4 = kc // 2  # rows 4*k4 .. 4*k4+3, 16 slots never wrap
                    base = ((4 * k4) % 16) * 68 * 1280
                    srcA = _ap(
                        bstage_all[:], base, [[1280, 36], [68 * 1280, 4], [1, 640]]
                    )
                    srcB = _ap(
                        bstage_all[:],
                        base + 32 * 1280 + 640,
                        [[1280, 36], [68 * 1280, 4], [1, 640]],
                    )
                    nc.sync.dma_start(
                        out=btX[0:36, 4 * k4 : 4 * k4 + 4, :], in_=srcA
                    )
                    nc.sync.dma_start(
                        out=btX[64:100, 4 * k4 : 4 * k4 + 4, :], in_=srcB
                    )

            # close A/Z/B psum pools to free banks for D
            psum_ab.close()
            psD = ctx.enter_context(tc.tile_pool(name="psD", bufs=3, space="PSUM"))

            # ---- stage D: banded reassembly + b_out ----
            # Each (h, ch-half) runs as 5 tap-passes x 2 concurrent half-row
            # matmuls on PE row-groups (0,0) and (64,0); the two w-halves
            # accumulate in separate PSUM banks of one 2-bank tile. All bias
            # adds on scalar; output DMAs batched 8 low-res rows on sync/scalar.
            obs = [None, None]
            for h in range(32):
                if h % 4 == 0:
                    obs[0] = opool.tile([128, 8, 64, 2], BF, tag="ob0", name="ob0")
                    obs[1] = opool.tile([128, 8, 64, 2], BF, tag="ob1", name="ob1")
                for half in range(2):
                    pd2 = psD.tile([128, 1024], FP, tag="D")
                    for i in range(5):
                        rhsA = _ap(
                            btX[:], h * 640 + 4 * i, [[20480, 36], [20, 32], [1, 4]]
                        )
                        nc.tensor.matmul(
                            pd2[:, 0:128].rearrange("p (w q) -> p w q", q=4),
                            zt[0:36, h + i, 128 * half : 128 * half + 128],
                            rhsA,
                            start=(i == 0), stop=(i == 4),
                        )
                        rhsB = _ap(
                            btX[:],
                            64 * 20480 + h * 640 + 4 * i,
                            [[20480, 36], [20, 32], [1, 4]],
                        )
                        nc.tensor.matmul(
                            pd2[:, 512:640].rearrange("p (w q) -> p w q", q=4),
                            zt2[64:100, h + i, 128 * half : 128 * half + 128],
                            rhsB,
                            start=(i == 0), stop=(i == 4),
                        )
                    ob = obs[half]
                    q = h % 4
                    for whalf in range(2):
                        pd_v = _ap(
                            pd2[:], 512 * whalf, [[1024, 128], [2, 2], [4, 32], [1, 2]]
                        )
                        dst = ob[:, 2 * q : 2 * q + 2, 32 * whalf : 32 * whalf + 32, :]
                        nc.scalar.add(dst, pd_v, add=bo[:, half : half + 1])
                if h % 4 == 3:
                    for half in range(2):
                        nc.sync.dma_start(
                            out=out_d[
                                128 * half : 128 * (half + 1),
                                2 * h - 6 : 2 * h + 2,
                                :,
                            ],
                            in_=obs[half][:].rearrange("p a w q -> p a (w q)"),
                        )

    nc.compile()
    return nc


def _host_prep(x, w_down, b_down, w_enc, b_enc, w_out, b_out):
    import ml_dtypes

    bft = ml_dtypes.bfloat16
    x = np.asarray(x, np.float32)
    xp = np.pad(x, [(0, 0), (0, 0), (2, 2), (2, 2)]).astype(bft)
    wdt = np.ascontiguousarray(np.asarray(w_down, np.float32)[:, :, 0, 0].T.astype(bft))
    wet = np.ascontiguousarray(
        np.asarray(w_enc, np.float32).transpose(1, 2, 3, 0).reshape(128, 9 * 100)
    ).astype(bft)
    wot = np.ascontiguousarray(np.asarray(w_out, np.float32)[:, :, 0, 0].T.astype(bft))
    # saug: permuted identity (e=(i5,j5,p4) -> e'=(j5,i5,p4)) + 4 group-sum cols
    saug = np.zeros((100, 104), bft)
    for i in range(5):
        for j in range(5):
            for p in range(4):
                saug[(i * 5 + j) * 4 + p, j * 20 + i * 4 + p] = 1.0
    for e in range(100):
        saug[e, 100 + e % 4] = 1.0
    wpk = np.zeros((128, W_COLS), bft)
    wpk[:, W_DT0 : W_DT0 + 128] = wdt[0:128]
    wpk[:, W_DT1 : W_DT1 + 128] = wdt[128:256]
    wpk[:, W_ET : W_ET + 900] = wet
    wpk[:, W_OT0 : W_OT0 + 256] = wot[0:128]
    wpk[:, W_OT1 : W_OT1 + 256] = wot[128:256]
    wpk[0:100, W_SAUG : W_SAUG + 104] = saug

    bd = np.asarray(b_down, np.float32).reshape(128)
    bev = np.asarray(b_enc, np.float32).reshape(100)
    bov = np.asarray(b_out, np.float32).reshape(256)
    in_maps = []
    for c in range(NCORES):
        n, hh = c // 2, c % 2
        xs = np.ascontiguousarray(xp[n, :, hh * 32 : hh * 32 + 36, :])
        bpk = np.zeros((128, 6), np.float32)
        bpk[:, 0] = bd
        bpk[0:100, 1] = bev
        bpk[:, 2] = bov[0:128]
        bpk[:, 3] = bov[128:256]
        bpk[:, 4] = 0.0 if hh == 0 else 1.0
        bpk[:, 5] = 0.0 if hh == 1 else 1.0
        in_maps.append(dict(xs=xs, wpk=wpk, bpk=bpk))
    return in_maps


last_exec_time_ns = None


def kernel(x, w_down, b_down, w_enc, b_enc, w_out, b_out):
    global last_exec_time_ns
    nc = _build()
    in_maps = _host_prep(x, w_down, b_down, w_enc, b_enc, w_out, b_out)
    res = run_bass_kernel_spmd(nc, in_maps, list(range(NCORES)))
    last_exec_time_ns = res.exec_time_ns
    out = np.empty((4, 256, 128, 128), np.float32)
    for c in range(NCORES):
        n, hh = c // 2, c % 2
        out[n, :, hh * 64 : (hh + 1) * 64, :] = np.asarray(
            res.results[c]["out"], dtype=np.float32
        )
    return out
